# revision 16
# baseline (speedup 1.0000x reference)
"""Fused LayerNorm + multi-head attention + output projection on 8 TRN2 NeuronCores.

Problem (hardcoded shapes): x [2, 2048, 1024] f32, 16 heads x 64 dim.
Sharding: data-parallel over batch (2) x tensor-parallel over head groups (4).
Core c handles batch c//4, heads [4*(c%4), 4*(c%4)+4). W_qkv column-sharded,
W_out row-sharded; per-core partial outputs are summed on the host.

Layout strategy (per core):
  - LayerNorm in [tok, dim], then PE-transpose to xnT [dim, tok] (bf16).
  - q,k are produced transposed (qT/kT [dhead, tok]) and DUPLICATED into both
    partition halves, so scores for two consecutive k-chunks run as two
    concurrent K=64 matmuls in different PE row-groups (double-pumped array).
  - scoresT [ktok, q] means softmax needs no transpose of the big matrix:
    exp on ACT (PSUM -> SBUF bf16), with the matmul scale folded in.
  - AV keeps V stationary ([v|ones] 65-col weights, one load per k-chunk) and
    streams expT; outT [65, q] accumulates in PSUM with softmax denominators
    in row 64.  Cheap per-q-tile PE back-transposes yield [q, 65] blocks whose
    reciprocal row scales the attention output (solves the sums layout).
  - Output projection keeps W_out stationary, emitting partial [dim, tok].
  - Emission is software-pipelined (AV lags scores/exp by 2 k-pairs; head
    drains ride inside the next head's stream) so the in-order PE queue never
    blocks on un-computed exponentials.
"""

import numpy as np
import ml_dtypes

B, N, DIM = 2, 2048, 1024
HEADS, DIM_HEAD = 16, 64
INNER = HEADS * DIM_HEAD
NCORES = 8
HG = 4                      # head-groups
HL = HEADS // HG            # heads per core (local)
QT = N // 128               # 16 q-tiles of 128 tokens
KC = N // 128               # 16 k-chunks of 128 tokens
KP = KC // 2                # 8 k-chunk pairs
DC = DIM // 128             # 8 dim chunks
TC4 = N // 512              # 4 chunks of 512 tokens
SCALE = DIM_HEAD ** -0.5
EPS = 1e-5

_cache = {}


def _build():
    import concourse.bass as bass
    import concourse.tile as tile
    from concourse import bacc, mybir

    f32 = mybir.dt.float32
    bf16 = mybir.dt.bfloat16
    AF = mybir.ActivationFunctionType

    nc = bacc.Bacc("TRN2", target_bir_lowering=False, debug=False,
                   num_devices=NCORES)

    x_d = nc.dram_tensor("x", [N, DIM], f32, kind="ExternalInput").ap()
    wqk_d = nc.dram_tensor("wqk", [DC, 128, 2 * HL * DIM_HEAD], bf16,
                           kind="ExternalInput").ap()
    wv_d = nc.dram_tensor("wv", [DC, 128, HL * DIM_HEAD], bf16,
                          kind="ExternalInput").ap()
    wout_d = nc.dram_tensor("wout", [2, 128, DIM], bf16,
                            kind="ExternalInput").ap()
    bqk_d = nc.dram_tensor("bqk", [128, 4], f32, kind="ExternalInput").ap()
    bv_d = nc.dram_tensor("bv", [128, 2], f32, kind="ExternalInput").ap()
    ident_d = nc.dram_tensor("ident", [128, 128], bf16,
                             kind="ExternalInput").ap()
    out_d = nc.dram_tensor("out", [DIM, N], f32, kind="ExternalOutput").ap()

    with tile.TileContext(nc) as tc:
        _graph(nc, tc, tile, bass, mybir, f32, bf16, AF,
               x_d, wqk_d, wv_d, wout_d, bqk_d, bv_d, ident_d, out_d)
    nc.compile()
    return nc


def _graph(nc, tc, tile, bass, mybir, f32, bf16, AF,
           x_d, wqk_d, wv_d, wout_d, bqk_d, bv_d, ident_d, out_d):
    from collections import deque
    from contextlib import ExitStack
    ctx = ExitStack()
    with ctx:
        # ---- persistent SBUF tensors -------------------------------------
        pers = ctx.enter_context(tc.tile_pool(name="pers", bufs=1))
        xnT = [pers.tile([128, DC, 512], bf16, tag=f"xnT{t}", name=f"xnT{t}")
               for t in range(TC4)]                              # 4 MB
        # duplicated-transposed q and k: [h][tchunk] -> [128, 512]
        q2 = [[pers.tile([128, 512], bf16, tag=f"q2_{h}_{t}",
                         name=f"q2_{h}_{t}") for t in range(TC4)]
              for h in range(HL)]
        k2 = [[pers.tile([128, 512], bf16, tag=f"k2_{h}_{t}",
                         name=f"k2_{h}_{t}") for t in range(TC4)]
              for h in range(HL)]
        v_ones = [pers.tile([128, 4, HL, DIM_HEAD + 1], bf16,
                            tag=f"vo{t}", name=f"vo{t}")
                  for t in range(TC4)]
        attn_s = pers.tile([128, QT, HL * DIM_HEAD], bf16, tag="attn")
        attnT = pers.tile([128, 2, N], bf16, tag="attnT")        # 1 MB
        wqk = pers.tile([128, DC, 512], bf16, tag="wqk")         # 1 MB
        wv = pers.tile([128, DC, 256], bf16, tag="wv")
        wout = pers.tile([128, 2, DIM], bf16, tag="wout")
        bqk = pers.tile([128, 4], f32, tag="bqk")
        bv = pers.tile([128, 2], f32, tag="bv")
        ident = pers.tile([128, 128], bf16, tag="ident")

        for d in range(DC):
            nc.sync.dma_start(wqk[:, d, :], wqk_d[d])
            nc.sync.dma_start(wv[:, d, :], wv_d[d])
        for i in range(2):
            nc.sync.dma_start(wout[:, i, :], wout_d[i])
        nc.sync.dma_start(bqk[:], bqk_d[:])
        nc.sync.dma_start(bv[:], bv_d[:])
        nc.sync.dma_start(ident[:], ident_d[:])
        for t in range(TC4):
            nc.gpsimd.memset(v_ones[t][:], 1.0)

        # ---- pools -------------------------------------------------------
        sb_x = ctx.enter_context(tc.tile_pool(name="sb_x", bufs=5))
        sb_xn = ctx.enter_context(tc.tile_pool(name="sb_xn", bufs=4))
        sb_st = ctx.enter_context(tc.tile_pool(name="sb_st", bufs=12))
        sb_exp = ctx.enter_context(tc.tile_pool(name="sb_exp", bufs=8))
        sb_u = ctx.enter_context(tc.tile_pool(name="sb_u", bufs=2))
        sb_o = ctx.enter_context(tc.tile_pool(name="sb_o", bufs=4))
        ps_a = ctx.enter_context(tc.tile_pool(name="ps_a", bufs=2,
                                              space="PSUM"))
        ps_o = ctx.enter_context(tc.tile_pool(name="ps_o", bufs=1,
                                              space="PSUM"))

        # ---- phase 2: attention, software-pipelined emission -------------
        # Emitted as (jp, qc) cells: scores for a k-chunk pair (two concurrent
        # K=64 matmuls in different PE row-groups) -> one exp -> deferred AV.
        # Head 0's cells are interleaved INTO phase 1 (wavefront) as their
        # q/k chunks become available; heads 1-3 follow, with each head's
        # drain riding inside the next head's stream.
        deferred = deque()
        po_t = {}

        def emit_av(h, jp, qc, et):
            if h not in po_t:
                po_t[h] = ps_o.tile([65, N], f32, tag="O", name=f"po{h}")
            po = po_t[h]
            for i in range(2):
                k = 2 * jp + i
                nc.tensor.matmul(po[:, qc * 512:(qc + 1) * 512],
                                 v_ones[k // 4][:, k % 4, h, :],
                                 et[:, i, :],
                                 start=(jp == 0 and i == 0),
                                 stop=(jp == KP - 1 and i == 1))

        def emit_drain(h):
            po = po_t.pop(h)
            ou = sb_u.tile([65, N], bf16, tag="u")
            nc.vector.tensor_copy(ou[:], po[:])
            if h == HL - 1:
                _tail(ou)
                return
            pbt = ps_a.tile([128, QT, 128], bf16, tag="A")
            for qt in range(QT):
                nc.tensor.transpose(pbt[:, qt, 0:65],
                                    ou[:, qt * 128:(qt + 1) * 128],
                                    ident[0:65, 0:65])
            for qt in range(QT):
                rec = sb_st.tile([128, 1], f32, tag="rec")
                nc.vector.reciprocal(rec[:], pbt[:, qt, 64:65])
                nc.vector.tensor_scalar_mul(
                    attn_s[:, qt, h * DIM_HEAD:(h + 1) * DIM_HEAD],
                    pbt[:, qt, 0:DIM_HEAD], rec[:])

        def _tail(ou):
            # last head's scale + attn transpose + output projection,
            # pipelined per 512-token chunk
            h = HL - 1
            for tp in range(TC4):
                pbt = ps_a.tile([128, 4, 128], bf16, tag="A",
                                name=f"pbt3_{tp}")
                for j in range(4):
                    qt = tp * 4 + j
                    nc.tensor.transpose(pbt[:, j, 0:65],
                                        ou[:, qt * 128:(qt + 1) * 128],
                                        ident[0:65, 0:65])
                for j in range(4):
                    qt = tp * 4 + j
                    rec = sb_st.tile([128, 1], f32, tag="rec")
                    nc.vector.reciprocal(rec[:], pbt[:, j, 64:65])
                    nc.vector.tensor_scalar_mul(
                        attn_s[:, qt, h * DIM_HEAD:(h + 1) * DIM_HEAD],
                        pbt[:, j, 0:DIM_HEAD], rec[:])
                pt2 = ps_a.tile([128, 4, 256], bf16, tag="A",
                                name=f"pt2_{tp}")
                for j in range(4):
                    qt = tp * 4 + j
                    for i in range(2):
                        nc.tensor.transpose(
                            pt2[:, j, i * 128:(i + 1) * 128],
                            attn_s[:, qt, i * 128:(i + 1) * 128], ident[:])
                for j in range(4):
                    qt = tp * 4 + j
                    for i in range(2):
                        nc.vector.tensor_scalar_add(
                            attnT[:, i, qt * 128:(qt + 1) * 128],
                            pt2[:, j, i * 128:(i + 1) * 128], bv[:, i:i + 1])
                for dcc in range(DC):
                    po2 = ps_a.tile([128, 512], f32, tag="A",
                                    name=f"po2_{tp}_{dcc}")
                    for i in range(2):
                        nc.tensor.matmul(
                            po2[:],
                            wout[:, i, dcc * 128:(dcc + 1) * 128],
                            attnT[:, i, tp * 512:(tp + 1) * 512],
                            start=(i == 0), stop=(i == 1))
                    ot = sb_o.tile([128, 512], f32, tag="o")
                    if dcc % 2 == 0:
                        nc.vector.tensor_copy(ot[:], po2[:])
                    else:
                        nc.scalar.copy(ot[:], po2[:])
                    nc.sync.dma_start(
                        out_d[dcc * 128:(dcc + 1) * 128,
                              tp * 512:(tp + 1) * 512],
                        ot[:])

        def flush_one():
            task = deferred.popleft()
            if task[0] == "av":
                emit_av(task[1], task[2], task[3], task[4])
            else:
                emit_drain(task[1])

        def emit_cell(h, jp, qc):
            ke, ko = 2 * jp, 2 * jp + 1
            tcq = ke // 4
            et = sb_exp.tile([128, 2, 512], bf16, tag="e")
            pscr = ps_a.tile([128, 1024], f32, tag="A")
            nc.tensor.matmul(
                pscr[:, 0:512],
                k2[h][tcq][0:64, (ke % 4) * 128:(ke % 4) * 128 + 128],
                q2[h][qc][0:64, :],
                start=True, stop=True)
            nc.tensor.matmul(
                pscr[:, 512:1024],
                k2[h][tcq][64:128, (ko % 4) * 128:(ko % 4) * 128 + 128],
                q2[h][qc][64:128, :],
                start=True, stop=True)
            nc.scalar.activation(
                et[:, :, :],
                pscr[:].rearrange("p (a b) -> p a b", a=2),
                AF.Exp, scale=SCALE)
            deferred.append(("av", h, jp, qc, et))
            while len(deferred) > 4:
                flush_one()

        def wavefront_cells(tc):
            cells = []
            for jp in (tc * 2, tc * 2 + 1):
                for qc in range(tc + 1):
                    cells.append((jp, qc))
            for jp in range(tc * 2):
                cells.append((jp, tc))
            cells.sort(key=lambda c: (c[1], c[0]))
            return cells

        def _emit_v(tchunk):
            for j in range(4):
                pv = ps_a.tile([128, 256], f32, tag="A", name=f"pv{tchunk}_{j}")
                for d in range(DC):
                    nc.tensor.matmul(pv[:],
                                     xnT[tchunk][:, d, j * 128:(j + 1) * 128],
                                     wv[:, d, :], start=(d == 0),
                                     stop=(d == DC - 1))
                nc.vector.tensor_copy(
                    v_ones[tchunk][:, j, :, 0:DIM_HEAD],
                    pv[:].rearrange("p (a b) -> p a b", a=HL))

        # ---- phase 1: LN -> xnT, q2/k2, v (per 512-token chunk) ----------
        for tchunk in range(TC4):
            for j in range(4):
                t = tchunk * 4 + j
                xt = sb_x.tile([128, DIM], f32, tag="x")
                nc.sync.dma_start(xt[:], x_d[t * 128:(t + 1) * 128, :])
                st6 = sb_st.tile([128, 2, 6], f32, tag="st6")
                nc.vector.bn_stats(st6[:, 0, :], xt[:, 0:512])
                nc.vector.bn_stats(st6[:, 1, :], xt[:, 512:1024])
                mv = sb_st.tile([128, 2], f32, tag="mv")
                nc.vector.bn_aggr(mv[:], st6[:].rearrange("p a b -> p (a b)"))
                veps = sb_st.tile([128, 1], f32, tag="ve")
                nc.vector.tensor_scalar_add(veps[:], mv[:, 1:2], EPS)
                sd = sb_st.tile([128, 1], f32, tag="sd")
                nc.scalar.activation(sd[:], veps[:], AF.Sqrt, bias=0.0)
                rs = sb_st.tile([128, 1], f32, tag="rs")
                nc.vector.reciprocal(rs[:], sd[:])
                nbias = sb_st.tile([128, 1], f32, tag="nb")
                nc.vector.tensor_mul(nbias[:], mv[:, 0:1], rs[:])
                nc.vector.tensor_scalar_mul(nbias[:], nbias[:], -1.0)
                xn = sb_xn.tile([128, DIM], bf16, tag="xn")
                nc.scalar.activation(xn[:], xt[:], AF.Identity,
                                     bias=nbias[:], scale=rs[:])
                for half in range(2):
                    pt = ps_a.tile([128, 512], bf16, tag="A")
                    for jj in range(4):
                        dcj = half * 4 + jj
                        nc.tensor.transpose(pt[:, jj * 128:(jj + 1) * 128],
                                            xn[:, dcj * 128:(dcj + 1) * 128],
                                            ident[:])
                    nc.vector.tensor_copy(
                        xnT[tchunk][:, half * 4:(half + 1) * 4,
                                    j * 128:(j + 1) * 128],
                        pt[:].rearrange("p (a b) -> p a b", a=4))
            # q/k (transposed, duplicated halves) for this token chunk
            for c in range(4):
                pq = ps_a.tile([128, 512], f32, tag="A")
                for d in range(DC):
                    nc.tensor.matmul(pq[:], wqk[:, d, c * 128:(c + 1) * 128],
                                     xnT[tchunk][:, d, :],
                                     start=(d == 0), stop=(d == DC - 1))
                dst = q2 if c < 2 else k2
                hA, hB = (c % 2) * 2, (c % 2) * 2 + 1
                nc.vector.tensor_scalar_add(dst[hA][tchunk][0:64, :],
                                            pq[0:64, :], bqk[0:64, c:c + 1])
                nc.vector.tensor_scalar_add(dst[hB][tchunk][0:64, :],
                                            pq[64:128, :],
                                            bqk[64:128, c:c + 1])
                nc.vector.tensor_copy(dst[hA][tchunk][64:128, :],
                                      dst[hA][tchunk][0:64, :])
                nc.vector.tensor_copy(dst[hB][tchunk][64:128, :],
                                      dst[hB][tchunk][0:64, :])
            _emit_v(tchunk)
            if tchunk > 0:
                for jp, qc in wavefront_cells(tchunk - 1):
                    emit_cell(0, jp, qc)

        # ---- phase 2 tail: finish head 0, then heads 1-3 -----------------
        for jp, qc in wavefront_cells(TC4 - 1):
            emit_cell(0, jp, qc)
        deferred.append(("drain", 0))
        for h in range(1, HL):
            for jp in range(KP):
                for qc in range(4):
                    emit_cell(h, jp, qc)
            deferred.append(("drain", h))
        while deferred:
            flush_one()


def _host_inputs(x, ln_gamma, ln_beta, W_qkv):
    """Per-core input maps (weights gamma-folded, bf16, head-group sharded)."""
    Wg = (ln_gamma[:, None] * W_qkv).astype(np.float32)
    beta_full = (ln_beta @ W_qkv).astype(np.float32)
    in_maps = []
    for c in range(NCORES):
        b, hg = c // HG, c % HG
        qcols = slice(256 * hg, 256 * hg + 256)
        kcols = slice(INNER + 256 * hg, INNER + 256 * hg + 256)
        vcols = slice(2 * INNER + 256 * hg, 2 * INNER + 256 * hg + 256)
        wqk = np.concatenate([Wg[:, qcols], Wg[:, kcols]], axis=1)
        wv = Wg[:, vcols]
        bqk = np.concatenate([beta_full[qcols], beta_full[kcols]])
        bvv = beta_full[vcols]
        in_maps.append({
            "x": np.ascontiguousarray(x[b], dtype=np.float32),
            "wqk": np.ascontiguousarray(
                wqk.reshape(DC, 128, 512)).astype(ml_dtypes.bfloat16),
            "wv": np.ascontiguousarray(
                wv.reshape(DC, 128, 256)).astype(ml_dtypes.bfloat16),
            "wout": None,  # filled by caller (needs W_out)
            "bqk": np.ascontiguousarray(
                bqk.reshape(4, 128).T).astype(np.float32),
            "bv": np.ascontiguousarray(
                bvv.reshape(2, 128).T).astype(np.float32),
            "ident": np.eye(128, dtype=np.float32).astype(ml_dtypes.bfloat16),
        })
    return in_maps


def kernel(x, ln_gamma, ln_beta, W_qkv, W_out, b_out):
    from concourse.bass_utils import run_bass_kernel_spmd

    if "nc" not in _cache:
        _cache["nc"] = _build()
    nc = _cache["nc"]

    x = np.asarray(x, dtype=np.float32)
    ln_gamma = np.asarray(ln_gamma, dtype=np.float32)
    ln_beta = np.asarray(ln_beta, dtype=np.float32)
    W_qkv = np.asarray(W_qkv, dtype=np.float32)
    W_out = np.asarray(W_out, dtype=np.float32)
    b_out = np.asarray(b_out, dtype=np.float32)

    in_maps = _host_inputs(x, ln_gamma, ln_beta, W_qkv)
    for c in range(NCORES):
        hg = c % HG
        wo = W_out[256 * hg:256 * hg + 256, :]
        in_maps[c]["wout"] = np.ascontiguousarray(
            wo.reshape(2, 128, DIM)).astype(ml_dtypes.bfloat16)

    res = run_bass_kernel_spmd(nc, in_maps, core_ids=list(range(NCORES)))
    kernel._last_results = res

    out = np.empty((B, N, DIM), dtype=np.float32)
    for b in range(B):
        acc = np.zeros((DIM, N), dtype=np.float32)
        for hg in range(HG):
            acc += res.results[b * HG + hg]["out"]
        out[b] = acc.T + b_out
    return out
